# revision 2
# baseline (speedup 1.0000x reference)
"""nn_LMAB kernel: data-parallel over batch across 8 NeuronCores.

Self-contained. Accepts FULL inputs (x: (16,256,40,40) f32 + params pytree),
returns FULL output. Shards batch 16 -> 8 devices x 2 samples via jax.pmap
on the Neuron (axon PJRT) devices; the block itself is XLA-compiled per core.
"""
import numpy as np
import jax
import jax.numpy as jnp
from functools import partial

C = 256; HEADS = 8; HD = C // HEADS
KS = (3, 5, 7); MA_DIL = (1, 2, 2)
HPR = (5, 2, 1)
HIDDEN = 512
DW_SIZES = (1, 3, 5, 7)
DW_CH = (HIDDEN - (HIDDEN // 4) * 3, HIDDEN // 4, HIDDEN // 4, HIDDEN // 4)
SMA_K = 3; SMA_DIL = (1, 2, 2)
EPS = 1e-6


def _ln2d(x, w, b):
    mu = x.mean(axis=1, keepdims=True)
    var = ((x - mu) ** 2).mean(axis=1, keepdims=True)
    return (x - mu) * jax.lax.rsqrt(var + EPS) * w[None, :, None, None] + b[None, :, None, None]


def _conv1x1(x, w, b):
    return jnp.einsum('oc,bchw->bohw', w, x) + b[None, :, None, None]


def _dwconv(x, w, b, dil=1):
    k = w.shape[-1]; p = (k // 2) * dil
    y = jax.lax.conv_general_dilated(x, w, (1, 1), [(p, p), (p, p)],
                                     rhs_dilation=(dil, dil),
                                     feature_group_count=x.shape[1],
                                     dimension_numbers=('NCHW', 'OIHW', 'NCHW'))
    return y + b[None, :, None, None]


def _unfold(x, ks, dil):
    B, Ch, H, W = x.shape
    p = (ks // 2) * dil
    xp = jnp.pad(x, ((0, 0), (0, 0), (p, p), (p, p)))
    return jnp.stack([xp[:, :, i * dil:i * dil + H, j * dil:j * dil + W]
                      for i in range(ks) for j in range(ks)], axis=2)


def _attend_range(q, k, v, ks, dil, rpb):
    B, h, d, H, W = q.shape
    kn = _unfold(k.reshape(B, h * d, H, W), ks, dil).reshape(B, h, d, ks * ks, H, W)
    vn = _unfold(v.reshape(B, h * d, H, W), ks, dil).reshape(B, h, d, ks * ks, H, W)
    logits = jnp.einsum('bhdxy,bhdlxy->bhlxy', q * (d ** -0.5), kn)
    logits = logits + rpb[None, :, :, None, None]
    attn = jax.nn.softmax(logits, axis=2)
    return jnp.einsum('bhlxy,bhdlxy->bhdxy', attn, vn)


def _attn(x, p):
    B, _, H, W = x.shape
    qkv = _conv1x1(x, p['qkv_w'], p['qkv_b'])
    q, k, v = jnp.split(qkv, 3, axis=1)
    q = q.reshape(B, HEADS, HD, H, W)
    k = k.reshape(B, HEADS, HD, H, W)
    v = v.reshape(B, HEADS, HD, H, W)
    outs = []
    hs = 0
    for i in range(len(KS)):
        he = hs + HPR[i]
        outs.append(_attend_range(q[:, hs:he], k[:, hs:he], v[:, hs:he],
                                  KS[i], MA_DIL[i], p['rpb'][i]))
        hs = he
    out = jnp.concatenate(outs, axis=1).reshape(B, C, H, W)
    return _conv1x1(out, p['proj_w'], p['proj_b'])


def _gelu(x):
    return jax.nn.gelu(x, approximate=False)


def _msconvstar(x, p):
    h = _conv1x1(x, p['fc1_w'], p['fc1_b'])
    parts = []
    s = 0
    for ch, w, b in zip(DW_CH, p['dw_w'], p['dw_b']):
        parts.append(_dwconv(h[:, s:s + ch], w, b))
        s += ch
    h = h + jnp.concatenate(parts, axis=1)
    h1, h2 = jnp.split(h, 2, axis=1)
    return _conv1x1(_gelu(h1) * h2, p['fc2_w'], p['fc2_b'])


def _cascaded_sma(x, p):
    for w, b, d in zip(p['dw_w'], p['dw_b'], SMA_DIL):
        x = _gelu(_dwconv(x, w, b, dil=d))
    return _conv1x1(x, p['pw_w'], p['pw_b'])


def _lmab(x, p):
    x = x + _attn(_ln2d(x, p['ln1_w'], p['ln1_b']), p)
    x = x + _msconvstar(_ln2d(x, p['ln2_w'], p['ln2_b']), p['mlp1'])
    x = x + _cascaded_sma(_ln2d(x, p['ln3_w'], p['ln3_b']), p['sma'])
    x = x + _msconvstar(_ln2d(x, p['ln4_w'], p['ln4_b']), p['mlp2'])
    return x


_PMAP = None


def _get_pmap():
    global _PMAP
    if _PMAP is None:
        _PMAP = jax.pmap(_lmab, axis_name='i', in_axes=(0, None),
                         devices=jax.devices()[:8])
    return _PMAP


def kernel(x, params):
    x = np.asarray(x, np.float32)
    B = x.shape[0]
    xs = x.reshape(8, B // 8, *x.shape[1:])
    f = _get_pmap()
    out = f(xs, params)
    out = np.asarray(out, np.float32).reshape(B, *x.shape[1:])
    return out
